# revision 9
# baseline (speedup 1.0000x reference)
"""Trainium2 Bass kernel for the controlled-unitary problem.

reference semantics (control=0, num_qubits=13, dim=8192):
    mask bit = 1 << 12, so columns/rows with that bit set are idx 4096..8191.
    out[:, c0] = state[:, c0]                       (control bit off: untouched)
    out[:, c1] = state[:, c1] @ target[c1, c1]      (controlled unitary)

Device work: complex [256,4096] @ [4096,4096] GEMM, Gauss 3-mult.
Sharding: output columns split 8 ways ([4096, 512] slab per core).

v3 design (per core):
  - Host sends planes ar, an=-ai, bd=bi-br, bs=br+bi packed per k-tile
    into TWO DRAM tensors (even/odd k-tiles), one per HWDGE ring.
    Row layout per partition: [ar(256) | an(256) | bd(512) | bs(512)].
  - Products: k2 = ar.bd and k3n = an.bs need NO device prep;
    j1 = (ar-an).(bs-bd) = 2*k1 needs two plain DVE subtracts per step.
    Combine: Cr = 0.5*j1 + k3n, Ci = 0.5*j1 + k2 (scale folded into the
    ACT PSUM->SBUF copy).
  - One big SBUF tile per ring (subtile deps), DMA in 9 ramped steps per
    ring, everything live (no slot recycling), outputs in fp16.
  - Last step emits m0's matmuls first so m0's combine + store overlap
    m1's final matmuls.
  - ~30 dummy N=128 matmuls on a zeroed scratch tile right after the
    preamble warm the PE HAM clock gate (4/8 -> 8/8) during the first
    DMA wait, so the real stream runs at the 216 ns/MM warm floor from
    matmul #0 instead of paying ~6 us of cold-clock ramp.
"""

import os

import numpy as np

BATCH = 256
DIM = 8192
HALF = 4096
N_CORES = 8
NSH = HALF // N_CORES  # 512 output columns per core
KT = HALF // 128  # 32 k-tiles
MT = BATCH // 128  # 2 m-tiles
KT_R = KT // 2  # 16 k-tiles per ring
# per-ring DMA step sizes (k-tiles per dma_start); the PE warm-up hides
# the first transfer, so steps are sized for DMA efficiency (>=0.75MB)
STEPS = [2, 2, 4, 4, 4]
assert sum(STEPS) == KT_R
ROW = 2 * BATCH + 2 * NSH  # 1536 packed elements per k-tile per partition

DT_NAME = "float16"  # kept for test.py compat

_CACHE = {}


def _build(dt_name="float16"):
    import concourse.mybir as mybir
    import concourse.tile as tile
    from concourse import bacc

    DT = mybir.dt.float16
    F32 = mybir.dt.float32

    nc = bacc.Bacc("TRN2", target_bir_lowering=False, debug=False,
                   num_devices=N_CORES)

    in0 = nc.dram_tensor("in0", [128, KT_R, ROW], DT, kind="ExternalInput")
    in1 = nc.dram_tensor("in1", [128, KT_R, ROW], DT, kind="ExternalInput")
    c_r = nc.dram_tensor("c_r", [BATCH, NSH], DT, kind="ExternalOutput")
    c_i = nc.dram_tensor("c_i", [BATCH, NSH], DT, kind="ExternalOutput")

    # packed row segment offsets
    AR0 = 0
    AN0 = BATCH
    BD0 = 2 * BATCH
    BS0 = 2 * BATCH + NSH

    with tile.TileContext(nc) as tc:
        with (
            tc.tile_pool(name="inp", bufs=1) as in_pool,
            tc.tile_pool(name="prep", bufs=1) as pr_pool,
            tc.tile_pool(name="op", bufs=1) as o_pool,
            tc.tile_pool(name="ps", bufs=1, space="PSUM") as ps_pool,
        ):
            warm = in_pool.tile([128, 256], DT, name="warm")
            it = [
                in_pool.tile([128, KT_R, ROW], DT, name=f"it{r}")
                for r in range(2)
            ]
            as_t = [
                pr_pool.tile([128, KT_R, BATCH], DT, name=f"as{r}")
                for r in range(2)
            ]
            bsub_t = [
                pr_pool.tile([128, KT_R, NSH], DT, name=f"bsub{r}")
                for r in range(2)
            ]
            ps = {}
            for m in range(MT):
                for comp in ("j1", "k2", "k3"):
                    ps[(m, comp)] = ps_pool.tile(
                        [128, NSH], F32, name=f"ps_{m}_{comp}"
                    )

            dram = (in0, in1)
            ring = (nc.sync, nc.scalar)

            # PE warm-up: dummy matmuls on zeroed scratch into a spare
            # PSUM bank, no DMA deps -> they run during the first DMA
            # wait and lift the HAM clock gate before real work lands
            N_WARM = 34
            ps_warm = ps_pool.tile([128, 512], F32, name="ps_warm")
            nc.gpsimd.memset(warm[:], 0.0)
            for w in range(N_WARM):
                nc.tensor.matmul(
                    ps_warm[:, :128], warm[:, :128], warm[:, 128:],
                    start=(w == 0), stop=(w == N_WARM - 1),
                )

            def emit_mms(kt, m, comps=("k2", "k3", "j1")):
                r = kt % 2
                pos = kt // 2
                msl = {  # lhsT slice per product
                    "k2": slice(AR0 + m * 128, AR0 + (m + 1) * 128),
                    "k3": slice(AN0 + m * 128, AN0 + (m + 1) * 128),
                }
                rhs = {
                    "k2": it[r][:, pos, BD0:BD0 + NSH],
                    "k3": it[r][:, pos, BS0:BS0 + NSH],
                    "j1": bsub_t[r][:, pos, :],
                }
                for comp in comps:
                    if comp == "j1":
                        lhsT = as_t[r][:, pos, m * 128:(m + 1) * 128]
                    else:
                        lhsT = it[r][:, pos, msl[comp]]
                    nc.tensor.matmul(
                        ps[(m, comp)][:], lhsT, rhs[comp],
                        start=(kt == 0), stop=(kt == KT - 1),
                    )

            p0 = 0
            for si, nk in enumerate(STEPS):
                psl = slice(p0, p0 + nk)
                last = si == len(STEPS) - 1
                kts = [2 * pos + r
                       for pos in range(p0, p0 + nk) for r in range(2)]
                for r in range(2):
                    ring[r].dma_start(it[r][:, psl, :], dram[r][:, psl, :])
                # j1 operand prep: as = ar - an, bsub = bs - bd
                for r in range(2):
                    nc.vector.tensor_tensor(
                        as_t[r][:, psl, :], it[r][:, psl, AR0:AR0 + BATCH],
                        it[r][:, psl, AN0:AN0 + BATCH],
                        mybir.AluOpType.subtract)
                    nc.vector.tensor_tensor(
                        bsub_t[r][:, psl, :], it[r][:, psl, BS0:BS0 + NSH],
                        it[r][:, psl, BD0:BD0 + NSH],
                        mybir.AluOpType.subtract)
                if last:
                    # m-major, j1 first within each m: m0's combine starts
                    # while m1's matmuls run, and m1's j1 bank completes
                    # ~8 matmuls early so its PSUM->SBUF scale overlaps too
                    for m in range(MT):
                        for comp in ("j1", "k2", "k3"):
                            for kt in kts:
                                emit_mms(kt, m, comps=(comp,))
                else:
                    # k2/k3 first (no prep dependency), all j1 last so the
                    # DVE prep has the whole k2/k3 stretch to complete
                    for comp in ("k2", "k3", "j1"):
                        for kt in kts:
                            for m in range(MT):
                                emit_mms(kt, m, comps=(comp,))
                p0 += nk

            for m in range(MT):
                msl = slice(m * 128, (m + 1) * 128)
                t1 = o_pool.tile([128, NSH], DT, name=f"t1_{m}")
                out_r = o_pool.tile([128, NSH], DT, name=f"or_{m}")
                out_i = o_pool.tile([128, NSH], DT, name=f"oi_{m}")
                # t1 = 0.5*j1 (PSUM -> SBUF; DVE, avoids ACT table load)
                nc.vector.tensor_scalar_mul(t1[:], ps[(m, "j1")][:], 0.5)
                nc.vector.tensor_tensor(out_r[:], ps[(m, "k3")][:], t1[:],
                                        mybir.AluOpType.add)
                nc.vector.tensor_tensor(out_i[:], ps[(m, "k2")][:], t1[:],
                                        mybir.AluOpType.add)
                nc.sync.dma_start(c_r[msl, :], out_r[:])
                nc.scalar.dma_start(c_i[msl, :], out_i[:])

    nc.compile()
    return nc


def _get_nc(dt_name=DT_NAME):
    if dt_name not in _CACHE:
        _CACHE[dt_name] = _build(dt_name)
    return _CACHE[dt_name]


def _pack_core(ar, an, bd, bs):
    """ar/an: [4096, 256] f16, bd/bs: [4096, 512] f16 ->
    (in0, in1) each [128, 16, 1536] (even/odd k-tiles)."""
    full = np.concatenate([
        ar.reshape(KT, 128, BATCH),
        an.reshape(KT, 128, BATCH),
        bd.reshape(KT, 128, NSH),
        bs.reshape(KT, 128, NSH),
    ], axis=2)  # [KT, 128, ROW]
    in0 = np.ascontiguousarray(full[0::2].transpose(1, 0, 2))
    in1 = np.ascontiguousarray(full[1::2].transpose(1, 0, 2))
    return in0, in1


def run_device(A, B, dt_name=DT_NAME, trace=False):
    """A: [256, 4096] complex64, B: [4096, 4096] complex64.
    Returns C = A @ B as [256, 4096] complex64 plus the raw results."""
    from concourse import bass_utils

    nc = _get_nc(dt_name)

    at = A.T  # [4096, 256]
    ar = np.ascontiguousarray(at.real).astype(np.float16)
    an = np.ascontiguousarray(-at.imag).astype(np.float16)
    br_full = B.real
    bi_full = B.imag

    in_maps = []
    for c in range(N_CORES):
        csl = slice(c * NSH, (c + 1) * NSH)
        br = br_full[:, csl]
        bi = bi_full[:, csl]
        bd = (bi - br).astype(np.float16)
        bs = (br + bi).astype(np.float16)
        in0, in1 = _pack_core(ar, an, bd, bs)
        in_maps.append({"in0": in0, "in1": in1})

    res = bass_utils.run_bass_kernel_spmd(
        nc, in_maps, core_ids=list(range(N_CORES)), trace=trace
    )

    out = np.empty((BATCH, HALF), dtype=np.complex64)
    for c in range(N_CORES):
        csl = slice(c * NSH, (c + 1) * NSH)
        out.real[:, csl] = res.results[c]["c_r"]
        out.imag[:, csl] = res.results[c]["c_i"]
    return out, res


def kernel(state, target_matrix, control, num_qubits):
    state = np.asarray(state)
    target_matrix = np.asarray(target_matrix)
    control = int(control)
    num_qubits = int(num_qubits)
    dim = 1 << num_qubits

    assert state.shape == (BATCH, DIM) and dim == DIM, (
        "kernel hardcoded for [256, 8192]"
    )

    mask = 1 << (num_qubits - control - 1)
    idx = np.arange(dim)
    c1 = idx[(idx & mask) != 0]  # columns with control bit set

    if control == 0:
        A = state[:, HALF:]
        B = target_matrix[HALF:, HALF:]
    else:
        A = state[:, c1]
        B = target_matrix[np.ix_(c1, c1)]
    A = np.ascontiguousarray(A, dtype=np.complex64)
    B = np.ascontiguousarray(B, dtype=np.complex64)

    C, _ = run_device(A, B)

    out = state.astype(np.complex64, copy=True)
    out[:, c1] = C
    return out


if __name__ == "__main__":
    # quick numeric self-check against numpy on random data
    rng = np.random.default_rng(0)
    A = (rng.standard_normal((BATCH, HALF)) +
         1j * rng.standard_normal((BATCH, HALF))).astype(np.complex64) / 90.5
    B = (rng.standard_normal((HALF, HALF)) +
         1j * rng.standard_normal((HALF, HALF))).astype(np.complex64) / 90.5
    C, _ = run_device(A, B)
    ref = A @ B
    err = np.linalg.norm(C - ref) / np.linalg.norm(ref)
    print("rel err vs numpy:", err)


# revision 11
# speedup vs baseline: 1.0511x; 1.0511x over previous
"""Trainium2 Bass kernel for the controlled-unitary problem.

reference semantics (control=0, num_qubits=13, dim=8192):
    mask bit = 1 << 12, so columns/rows with that bit set are idx 4096..8191.
    out[:, c0] = state[:, c0]                       (control bit off: untouched)
    out[:, c1] = state[:, c1] @ target[c1, c1]      (controlled unitary)

Device work: complex [256,4096] @ [4096,4096] GEMM, Gauss 3-mult.
Sharding: output columns split 8 ways ([4096, 512] slab per core).

v3 design (per core):
  - Host sends planes ar, an=-ai, bd=bi-br, bs=br+bi packed per k-tile
    into TWO DRAM tensors (even/odd k-tiles), one per HWDGE ring.
    Row layout per partition: [ar(256) | an(256) | bd(512) | bs(512)].
  - Products: k2 = ar.bd and k3n = an.bs need NO device prep;
    j1 = (ar-an).(bs-bd) = 2*k1 needs two plain DVE subtracts per step.
    Combine: Cr = 0.5*j1 + k3n, Ci = 0.5*j1 + k2 (scale folded into the
    ACT PSUM->SBUF copy).
  - One big SBUF tile per ring (subtile deps), DMA in 9 ramped steps per
    ring, everything live (no slot recycling), outputs in fp16.
  - Last step emits m0's matmuls first so m0's combine + store overlap
    m1's final matmuls.
  - ~30 dummy N=128 matmuls on a zeroed scratch tile right after the
    preamble warm the PE HAM clock gate (4/8 -> 8/8) during the first
    DMA wait, so the real stream runs at the 216 ns/MM warm floor from
    matmul #0 instead of paying ~6 us of cold-clock ramp.
"""

import os

import numpy as np

BATCH = 256
DIM = 8192
HALF = 4096
N_CORES = 8
NSH = HALF // N_CORES  # 512 output columns per core
KT = HALF // 128  # 32 k-tiles
MT = BATCH // 128  # 2 m-tiles
KT_R = KT // 2  # 16 k-tiles per ring
# per-ring DMA step sizes (k-tiles per dma_start); small first steps so
# the first tiles land fast even on HBM-contended cores (the max core
# sets the score, and a post-warm-up PE idle >3.4us re-throttles HAM)
STEPS = [1, 1, 2, 2, 2, 2, 2, 2, 2]
assert sum(STEPS) == KT_R
ROW = 2 * BATCH + 2 * NSH  # 1536 packed elements per k-tile per partition

DT_NAME = "float16"  # kept for test.py compat

_CACHE = {}


def _build(dt_name="float16"):
    import concourse.mybir as mybir
    import concourse.tile as tile
    from concourse import bacc

    DT = mybir.dt.float16
    F32 = mybir.dt.float32

    nc = bacc.Bacc("TRN2", target_bir_lowering=False, debug=False,
                   num_devices=N_CORES)

    in0 = nc.dram_tensor("in0", [128, KT_R, ROW], DT, kind="ExternalInput")
    in1 = nc.dram_tensor("in1", [128, KT_R, ROW], DT, kind="ExternalInput")
    c_r = nc.dram_tensor("c_r", [BATCH, NSH], DT, kind="ExternalOutput")
    c_i = nc.dram_tensor("c_i", [BATCH, NSH], DT, kind="ExternalOutput")

    # packed row segment offsets
    AR0 = 0
    AN0 = BATCH
    BD0 = 2 * BATCH
    BS0 = 2 * BATCH + NSH

    with tile.TileContext(nc) as tc:
        with (
            tc.tile_pool(name="inp", bufs=1) as in_pool,
            tc.tile_pool(name="prep", bufs=1) as pr_pool,
            tc.tile_pool(name="op", bufs=1) as o_pool,
            tc.tile_pool(name="ps", bufs=1, space="PSUM") as ps_pool,
        ):
            warm = in_pool.tile([128, 256], DT, name="warm")
            it = [
                in_pool.tile([128, KT_R, ROW], DT, name=f"it{r}")
                for r in range(2)
            ]
            as_t = [
                pr_pool.tile([128, KT_R, BATCH], DT, name=f"as{r}")
                for r in range(2)
            ]
            bsub_t = [
                pr_pool.tile([128, KT_R, NSH], DT, name=f"bsub{r}")
                for r in range(2)
            ]
            ps = {}
            for m in range(MT):
                for comp in ("j1", "k2", "k3"):
                    ps[(m, comp)] = ps_pool.tile(
                        [128, NSH], F32, name=f"ps_{m}_{comp}"
                    )

            dram = (in0, in1)
            ring = (nc.sync, nc.scalar)

            # PE warm-up: dummy matmuls on zeroed scratch into a spare
            # PSUM bank, no DMA deps -> they run during the first DMA
            # wait and lift the HAM clock gate before real work lands
            N_WARM = 34
            ps_warm = ps_pool.tile([128, 512], F32, name="ps_warm")
            nc.gpsimd.memset(warm[:], 0.0)
            for w in range(N_WARM):
                nc.tensor.matmul(
                    ps_warm[:, :128], warm[:, :128], warm[:, 128:],
                    start=(w == 0), stop=(w == N_WARM - 1),
                )

            def emit_mms(kt, m, comps=("k2", "k3", "j1")):
                r = kt % 2
                pos = kt // 2
                msl = {  # lhsT slice per product
                    "k2": slice(AR0 + m * 128, AR0 + (m + 1) * 128),
                    "k3": slice(AN0 + m * 128, AN0 + (m + 1) * 128),
                }
                rhs = {
                    "k2": it[r][:, pos, BD0:BD0 + NSH],
                    "k3": it[r][:, pos, BS0:BS0 + NSH],
                    "j1": bsub_t[r][:, pos, :],
                }
                for comp in comps:
                    if comp == "j1":
                        lhsT = as_t[r][:, pos, m * 128:(m + 1) * 128]
                    else:
                        lhsT = it[r][:, pos, msl[comp]]
                    nc.tensor.matmul(
                        ps[(m, comp)][:], lhsT, rhs[comp],
                        start=(kt == 0), stop=(kt == KT - 1),
                    )

            p0 = 0
            for si, nk in enumerate(STEPS):
                psl = slice(p0, p0 + nk)
                last = si == len(STEPS) - 1
                kts = [2 * pos + r
                       for pos in range(p0, p0 + nk) for r in range(2)]
                for r in range(2):
                    if si == 0:
                        # split so the k2 operands (ar+an+bd) land first
                        ring[r].dma_start(it[r][:, psl, :BS0],
                                          dram[r][:, psl, :BS0])
                        ring[r].dma_start(it[r][:, psl, BS0:],
                                          dram[r][:, psl, BS0:])
                    else:
                        ring[r].dma_start(it[r][:, psl, :], dram[r][:, psl, :])
                # j1 operand prep: as = ar - an, bsub = bs - bd
                for r in range(2):
                    nc.vector.tensor_tensor(
                        as_t[r][:, psl, :], it[r][:, psl, AR0:AR0 + BATCH],
                        it[r][:, psl, AN0:AN0 + BATCH],
                        mybir.AluOpType.subtract)
                    nc.vector.tensor_tensor(
                        bsub_t[r][:, psl, :], it[r][:, psl, BS0:BS0 + NSH],
                        it[r][:, psl, BD0:BD0 + NSH],
                        mybir.AluOpType.subtract)
                if last:
                    # m-major, j1 first within each m: m0's combine starts
                    # while m1's matmuls run, and m1's j1 bank completes
                    # ~8 matmuls early so its PSUM->SBUF scale overlaps too
                    for m in range(MT):
                        for comp in ("j1", "k2", "k3"):
                            for kt in kts:
                                emit_mms(kt, m, comps=(comp,))
                else:
                    # k2/k3 first (no prep dependency), all j1 last so the
                    # DVE prep has the whole k2/k3 stretch to complete
                    for comp in ("k2", "k3", "j1"):
                        for kt in kts:
                            for m in range(MT):
                                emit_mms(kt, m, comps=(comp,))
                p0 += nk

            for m in range(MT):
                msl = slice(m * 128, (m + 1) * 128)
                t1 = o_pool.tile([128, NSH], DT, name=f"t1_{m}")
                out_r = o_pool.tile([128, NSH], DT, name=f"or_{m}")
                out_i = o_pool.tile([128, NSH], DT, name=f"oi_{m}")
                # t1 = 0.5*j1 (PSUM -> SBUF; DVE, avoids ACT table load)
                nc.vector.tensor_scalar_mul(t1[:], ps[(m, "j1")][:], 0.5)
                nc.vector.tensor_tensor(out_r[:], ps[(m, "k3")][:], t1[:],
                                        mybir.AluOpType.add)
                nc.vector.tensor_tensor(out_i[:], ps[(m, "k2")][:], t1[:],
                                        mybir.AluOpType.add)
                nc.sync.dma_start(c_r[msl, :], out_r[:])
                nc.scalar.dma_start(c_i[msl, :], out_i[:])

    nc.compile()
    return nc


def _get_nc(dt_name=DT_NAME):
    if dt_name not in _CACHE:
        _CACHE[dt_name] = _build(dt_name)
    return _CACHE[dt_name]


def _pack_core(ar, an, bd, bs):
    """ar/an: [4096, 256] f16, bd/bs: [4096, 512] f16 ->
    (in0, in1) each [128, 16, 1536] (even/odd k-tiles)."""
    full = np.concatenate([
        ar.reshape(KT, 128, BATCH),
        an.reshape(KT, 128, BATCH),
        bd.reshape(KT, 128, NSH),
        bs.reshape(KT, 128, NSH),
    ], axis=2)  # [KT, 128, ROW]
    in0 = np.ascontiguousarray(full[0::2].transpose(1, 0, 2))
    in1 = np.ascontiguousarray(full[1::2].transpose(1, 0, 2))
    return in0, in1


def run_device(A, B, dt_name=DT_NAME, trace=False):
    """A: [256, 4096] complex64, B: [4096, 4096] complex64.
    Returns C = A @ B as [256, 4096] complex64 plus the raw results."""
    from concourse import bass_utils

    nc = _get_nc(dt_name)

    at = A.T  # [4096, 256]
    ar = np.ascontiguousarray(at.real).astype(np.float16)
    an = np.ascontiguousarray(-at.imag).astype(np.float16)
    br_full = B.real
    bi_full = B.imag

    in_maps = []
    for c in range(N_CORES):
        csl = slice(c * NSH, (c + 1) * NSH)
        br = br_full[:, csl]
        bi = bi_full[:, csl]
        bd = (bi - br).astype(np.float16)
        bs = (br + bi).astype(np.float16)
        in0, in1 = _pack_core(ar, an, bd, bs)
        in_maps.append({"in0": in0, "in1": in1})

    res = bass_utils.run_bass_kernel_spmd(
        nc, in_maps, core_ids=list(range(N_CORES)), trace=trace
    )

    out = np.empty((BATCH, HALF), dtype=np.complex64)
    for c in range(N_CORES):
        csl = slice(c * NSH, (c + 1) * NSH)
        out.real[:, csl] = res.results[c]["c_r"]
        out.imag[:, csl] = res.results[c]["c_i"]
    return out, res


def kernel(state, target_matrix, control, num_qubits):
    state = np.asarray(state)
    target_matrix = np.asarray(target_matrix)
    control = int(control)
    num_qubits = int(num_qubits)
    dim = 1 << num_qubits

    assert state.shape == (BATCH, DIM) and dim == DIM, (
        "kernel hardcoded for [256, 8192]"
    )

    mask = 1 << (num_qubits - control - 1)
    idx = np.arange(dim)
    c1 = idx[(idx & mask) != 0]  # columns with control bit set

    if control == 0:
        A = state[:, HALF:]
        B = target_matrix[HALF:, HALF:]
    else:
        A = state[:, c1]
        B = target_matrix[np.ix_(c1, c1)]
    A = np.ascontiguousarray(A, dtype=np.complex64)
    B = np.ascontiguousarray(B, dtype=np.complex64)

    C, _ = run_device(A, B)

    out = state.astype(np.complex64, copy=True)
    out[:, c1] = C
    return out


if __name__ == "__main__":
    # quick numeric self-check against numpy on random data
    rng = np.random.default_rng(0)
    A = (rng.standard_normal((BATCH, HALF)) +
         1j * rng.standard_normal((BATCH, HALF))).astype(np.complex64) / 90.5
    B = (rng.standard_normal((HALF, HALF)) +
         1j * rng.standard_normal((HALF, HALF))).astype(np.complex64) / 90.5
    C, _ = run_device(A, B)
    ref = A @ B
    err = np.linalg.norm(C - ref) / np.linalg.norm(ref)
    print("rel err vs numpy:", err)
